# revision 17
# baseline (speedup 1.0000x reference)
"""Trainium2 Bass kernel for nn_BinaryDiff: out = x @ base + coeff * (x @ mask).

Fused as a single matmul: out = x @ W where W = base + coeff * mask.

Sharding over 8 NeuronCores: 4 row-groups of x (2048 rows each) x 2
column-groups of W (2048 cols each). Each core computes a [2048, 2048]
tile of the [8192, 4096] output.

Per-core device kernel (x^T-resident, W-streaming — single pass over x):
  - x is read from HBM once (1024-wide f32 slices on the sync/HWDGE
    queue, 4-deep ring), PE-transposed DIRECTLY from f32 (2 cyc/row;
    no cast stage, no ScalarE in the x path), f32 PSUM groups of 8
    k-tiles -> one merged DVE copyback with f32->bf16 conversion into
    the resident x^T cache [128, K/128, M] bf16 (128 KiB/partition).
  - W = base + c*mask is built on-chip (GpSimd scalar_tensor_tensor
    per slab, int32 mask consumed directly; base slabs on sync queue,
    mask slabs on gpsimd queue) and STREAMED in five column chunks
    [256, 448, 448, 448, 448] through a 2-deep ring; each chunk is
    consumed by a full pass over all 16 m-tiles with k-contiguous PSUM
    accumulation (32 matmuls per psum).
  - W slab emission is split issue-phase/compute-phase per burst so a
    waiting STT never head-of-line blocks later DMA issues (engine
    queues are FIFO).
  - Chunk 0 is narrow (256) so its W lands during the x stream; matmul
    emission lags x by LAG m-tiles so the PE transposes while chunk
    0's W loads. Later chunks' loads are paced onto the previous
    chunk's output copybacks via explicit deps.
  - Output copyback on ScalarE (delayed one m-tile); staging shares
    the "bst" ring (SBUF fully budgeted: 128k x^T + 56k W ring + 16k x
    ring + 7k W staging per partition).
"""

import numpy as np

import concourse.bass as bass
import concourse.mybir as mybir
import concourse.tile as tile
from concourse import bacc
from concourse.masks import make_identity

P = 128
FULL_M, FULL_K, FULL_N = 8192, 4096, 4096
ROW_SHARDS, COL_SHARDS = 4, 2
CORE_M = FULL_M // ROW_SHARDS   # 2048
CORE_N = FULL_N // COL_SHARDS   # 2048


def build_kernel(M=CORE_M, K=FULL_K, N=CORE_N, debug=False):
    """Build the per-core Bass program. All cores run the same program (SPMD)."""
    f32 = mybir.dt.float32
    i32 = mybir.dt.int32
    bf16 = mybir.dt.bfloat16

    M_T = M // P            # 16 m-tiles of 128 rows
    K_T = K // P            # 32 k-tiles of 128
    CHUNKS = [384, 416, 416, 416, 416]   # W column chunks (sum = N)
    assert sum(CHUNKS) == N
    COFF = [sum(CHUNKS[:q]) for q in range(len(CHUNKS))]
    NQ = len(CHUNKS)
    XS = 1024               # x staging slice width (f32)
    TG = XS // P            # 8 k-tiles per transpose group (= one x slice)
    NG = K_T // TG          # 4 transpose groups per m-tile
    LAG = 8                 # m-tiles of matmul lag behind x in chunk 0

    nc = bacc.Bacc("TRN2", target_bir_lowering=False, debug=debug)

    x_d = nc.dram_tensor("x", [M, K], f32, kind="ExternalInput").ap()
    base_d = nc.dram_tensor("base", [K, N], f32, kind="ExternalInput").ap()
    mask_d = nc.dram_tensor("mask", [K, N], i32, kind="ExternalInput").ap()
    coeff_d = nc.dram_tensor("coeff", [P, 1], f32, kind="ExternalInput").ap()
    out_d = nc.dram_tensor("out", [M, N], f32, kind="ExternalOutput").ap()

    with tile.TileContext(nc) as tc:
        with (
            tc.tile_pool(name="const", bufs=1) as const,
            tc.tile_pool(name="xtc", bufs=1) as xtc,
            tc.tile_pool(name="wpool", bufs=2) as wpool,
            tc.tile_pool(name="wstage", bufs=3) as wstage,
            tc.tile_pool(name="xstage", bufs=4) as xstage,
            tc.tile_pool(name="ostage", bufs=1) as ostage,
            tc.tile_pool(name="tpsum", bufs=2, space="PSUM") as tpsum,
            tc.tile_pool(name="mpsum", bufs=4, space="PSUM") as mpsum,
        ):
            ident = const.tile([P, P], f32)
            make_identity(nc, ident[:])
            c128 = const.tile([P, 1], f32)
            nc.sync.dma_start(out=c128[:], in_=coeff_d[:])

            # x^T cache: [128, kt, m] bf16, written once, read by all matmuls
            xt = xtc.tile([P, K_T, M], bf16, name="xt")

            WSW = max(CHUNKS)   # wstage slab slot width

            def build_w_burst(w, q, kts, anchor=None):
                """Load base/mask slabs for several k-tiles of chunk q, fuse
                into W. Issue ALL the DMAs first, then the STTs, so a
                data-waiting STT never blocks later DMA issues in the FIFO
                gpsimd queue. `anchor` paces the burst's DMAs."""
                cw = CHUNKS[q]
                cs = slice(COFF[q], COFF[q] + cw)
                slabs = []
                for kt in kts:
                    bst = wstage.tile([P, WSW], f32, name="bst")
                    mst = wstage.tile([P, WSW], i32, name="mst", bufs=2)
                    d1 = nc.sync.dma_start(
                        out=bst[:, 0:cw], in_=base_d[kt * P:(kt + 1) * P, cs]
                    )
                    d2 = nc.sync.dma_start(
                        out=mst[:, 0:cw], in_=mask_d[kt * P:(kt + 1) * P, cs]
                    )
                    if anchor is not None:
                        tile.add_dep_helper(d1.ins, anchor.ins, reason="pace W")
                        tile.add_dep_helper(d2.ins, anchor.ins, reason="pace W")
                    slabs.append((kt, bst, mst))
                for kt, bst, mst in slabs:
                    nc.vector.scalar_tensor_tensor(
                        out=w[:, kt, 0:cw],
                        in0=mst[:, 0:cw],
                        scalar=c128[:, 0:1],
                        in1=bst[:, 0:cw],
                        op0=mybir.AluOpType.mult,
                        op1=mybir.AluOpType.add,
                    )

            def emit_x_m(m):
                """Stage x rows f32 (sync/HWDGE), PE-transpose straight from
                f32 in groups of 8 k-tiles (f32 PSUM, 2 banks), merged DVE
                copyback with f32->bf16 conversion into xt."""
                rs = slice(m * P, (m + 1) * P)
                for g in range(NG):
                    xst = xstage.tile([P, XS], f32, name="xst")
                    nc.sync.dma_start(
                        out=xst[:], in_=x_d[rs, g * XS:(g + 1) * XS]
                    )
                    pst = tpsum.tile([P, TG, P], f32)
                    for j in range(TG):
                        nc.tensor.transpose(
                            pst[:, j, :], xst[:, j * P:(j + 1) * P], ident[:]
                        )
                    nc.vector.tensor_copy(
                        out=xt[:, g * TG:(g + 1) * TG, m * P:(m + 1) * P],
                        in_=pst[:, :, :],
                    )

            pending = [None]  # (psum, m, q) awaiting copyback + store

            def flush_pending():
                if pending[0] is None:
                    return None
                ps, m, q = pending[0]
                pending[0] = None
                cw = CHUNKS[q]
                ob = ostage.tile([P, WSW], f32, name="ob")
                cp = nc.scalar.copy(out=ob[:, 0:cw], in_=ps[:])
                nc.scalar.dma_start(
                    out=out_d[m * P:(m + 1) * P, COFF[q]:COFF[q] + cw],
                    in_=ob[:, 0:cw],
                )
                return cp

            w_cur = wpool.tile([P, K_T, WSW], bf16, name="wch")
            w_next = None

            for q in range(NQ):
                lag = LAG if q == 0 else 0
                if q + 1 < NQ:
                    w_next = wpool.tile([P, K_T, WSW], bf16, name="wch")
                    # prefetch pacing: anchor chunk q+1's slab loads onto
                    # chunk q's output copybacks. Chunk 0: back half (x
                    # nearly done); later chunks: spread from m=2.
                    nxt = {}
                    m0, span = (8, 8) if q == 0 else (1, 8)
                    for kt in range(K_T):
                        nxt.setdefault(m0 + (kt * span) // K_T, []).append(kt)
                else:
                    w_next = None
                    nxt = {}

                for i in range(M_T + lag):
                    # Emit the (always-ready) matmul group BEFORE the x phase
                    # so a late x slice never idles the PE with ready work
                    # stuck behind it in the FIFO (keeps HAM warm in chunk 0).
                    m = i - lag
                    if m >= 0:
                        anchor = flush_pending()
                        if anchor is not None and w_next is not None and m > 0:
                            kts = nxt.get(m - 1, [])
                            if kts:
                                build_w_burst(w_next, q + 1, kts, anchor=anchor)
                        ps = mpsum.tile([P, CHUNKS[q]], f32, name="mmps")
                        for kt in range(K_T):
                            nc.tensor.matmul(
                                ps[:],
                                lhsT=xt[:, kt, m * P:(m + 1) * P],
                                rhs=w_cur[:, kt, 0:CHUNKS[q]],
                                start=(kt == 0),
                                stop=(kt == K_T - 1),
                            )
                        pending[0] = (ps, m, q)
                    if q == 0 and i < M_T:
                        # JIT-interleave chunk-0 W builds with x phases so
                        # gpsimd (mask DMA + STT) flows in arrival order.
                        if i < 8:
                            build_w_burst(w_cur, 0, range(i * 4, i * 4 + 4))
                        emit_x_m(i)
                # any prefetch slabs whose anchor never flushed in-loop
                if w_next is not None:
                    rest = []
                    for mm_key, kts in sorted(nxt.items()):
                        if mm_key + 1 + lag > M_T + lag - 1:
                            rest.extend(kts)
                    if rest:
                        build_w_burst(w_next, q + 1, rest)
                    w_cur = w_next

            flush_pending()

    nc.compile()
    return nc


_NC_CACHE = {}


def _get_nc():
    if "nc" not in _NC_CACHE:
        _NC_CACHE["nc"] = build_kernel()
    return _NC_CACHE["nc"]


def make_in_maps(x, base, coeff, mask):
    x = np.asarray(x, dtype=np.float32)
    base = np.asarray(base, dtype=np.float32)
    mask = np.asarray(mask, dtype=np.int32)
    coeff = np.asarray(coeff, dtype=np.float32)

    B, L, D_IN = x.shape
    x2 = np.ascontiguousarray(x.reshape(B * L, D_IN))
    c128 = np.full((P, 1), coeff[0], dtype=np.float32)

    in_maps = []
    for i in range(8):
        rg, cg = i // COL_SHARDS, i % COL_SHARDS
        in_maps.append(
            {
                "x": x2[rg * CORE_M:(rg + 1) * CORE_M],
                "base": np.ascontiguousarray(
                    base[:, cg * CORE_N:(cg + 1) * CORE_N]
                ),
                "mask": np.ascontiguousarray(
                    mask[:, cg * CORE_N:(cg + 1) * CORE_N]
                ),
                "coeff": c128,
            }
        )
    return in_maps, (B, L)


def assemble(results, B, L):
    out = np.empty((B * L, FULL_N), dtype=np.float32)
    for i in range(8):
        rg, cg = i // COL_SHARDS, i % COL_SHARDS
        out[rg * CORE_M:(rg + 1) * CORE_M, cg * CORE_N:(cg + 1) * CORE_N] = (
            results[i]["out"]
        )
    return out.reshape(B, L, FULL_N)


def kernel(x, base, coeff, mask):
    from concourse.bass_utils import run_bass_kernel_spmd

    in_maps, (B, L) = make_in_maps(x, base, coeff, mask)
    nc = _get_nc()
    res = run_bass_kernel_spmd(nc, in_maps, list(range(8)))
    return assemble(res.results, B, L)
